# revision 53
# baseline (speedup 1.0000x reference)
"""Distributed AttentionAutoEncoder kernel for 8 TRN2 NeuronCores (Bass/Tile).

Reference computation (fp32):
    Q = W_q @ X ; K = W_v @ X ; V = W_k @ X          (d=2048, n=8192)
    S = (Q @ K.T) / sqrt(d) ; Z = softmax(S, -1) ; A = Z @ V

Reformulation: S = W_q G W_v'^T with G = X X^T (contracting n) and
W_v' = W_v^T/sqrt(d).  n is sharded 1024/core; the cross-core reduction
is pushed through the chain as ReduceScatters (only the scattered
output is charged by the interconnect) instead of AllReducing G:

  G_c = X_c X_c^T            full d x d, single-pass float32r (fp22)
  RS1 (2 halves, first issued mid-G): G rows {128c, 1024+128c} summed
  T^T[j,:] = sum_{k local} Wv'[k,j] G[k,:]      (fp32r)
  RS2 (2 halves, first issued mid-T): T^T rows j_c = {128c, 1024+128c}
  S[:, j_c] = W_q @ T[:, j_c]                   (fp32r, after 32 PE
        transposes of the T^T shard)
  softmax: local row maxes AllGathered in 2 halves (first hides under
        the S loop), reduced locally; exp to fp16 then PE-transposed
        and converted to fp8
  AG: P^T in two fp8 chunks (j-tiles c and 8+c) + a tiny AllGather of
        the local exp-sums, ordered P-a, P-b, sums so the A-loop's
        first half overlaps chunk b on the serial collective engine
  A = P V: gathered P^T tiles (fp8 -> fp16) are used directly as
        stationary operands - no transposes in the tail; the pass-a
        partial is drained to fp16 SBUF and folded back into pass-b's
        PSUM with an identity matmul; 1/rowsum is applied in the final
        per-row scale.

V = W_k X_c is data-parallel fp16, emitted in sweeps between the
phases so the in-order PE fills the collective latency windows.
G/T/S matmuls are single-pass float32r (1 cyc/row at moving >= 256);
V/A are fp16.  End-to-end fro error ~1.8e-3 on hardware vs the 2e-2
gate (the fp22 S chain dominates; fp8 P costs nothing extra).
"""

import numpy as np

import concourse.bacc as bacc
import concourse.mybir as mybir
import concourse.tile as tile
from concourse.masks import make_identity

P = 128
FP16 = mybir.dt.float16
FP32 = mybir.dt.float32
FP32R = mybir.dt.float32r
FP8 = mybir.dt.float8e4
AF = mybir.ActivationFunctionType

D_FULL = 2048
N_FULL = 8192
NCORES = 8


def build(D=D_FULL, NL=N_FULL // NCORES, NC=NCORES, stop_after=None,
          mock_coll=False, v_slices=(2, 2, 0)):
    """Build the SPMD Bass program (identical on every core)."""
    KS = D // NC          # k-rows of reduced G / j-rows of T^T per core
    JS = D // NC
    nT = NL // P          # 8
    dT = D // P           # 16
    CB = 512              # fp32 moving max
    KB = D // CB          # 4
    NBS = 512
    NB = NL // NBS        # 2
    JT = KS // P          # 2 j-tiles per core

    nc = bacc.Bacc("TRN2", target_bir_lowering=False, debug=False,
                   num_devices=NC)

    # ------------- I/O -------------
    xt_f32 = nc.dram_tensor("xt_f32", [NL, D], FP32R, kind="ExternalInput")
    xn_f16 = nc.dram_tensor("xn_f16", [D, NL], FP16, kind="ExternalInput")
    wkt_g = nc.dram_tensor("wkt_g", [dT // 4, dT, P, 4 * P], FP16,
                           kind="ExternalInput")
    wvts = nc.dram_tensor("wvts", [KS, D], FP32R, kind="ExternalInput")
    wqt_b = nc.dram_tensor("wqt_b", [dT, P, dT, P], FP32R,
                           kind="ExternalInput")
    a_out = nc.dram_tensor("a_out", [D, NL], FP32, kind="ExternalOutput")

    with tile.TileContext(nc) as tc:
        with tc.tile_pool(name="dram", bufs=1, space="DRAM") as dpool:
            g_in = dpool.tile([D, D], FP32R, name="g_in")
            g_rs1 = dpool.tile([P, D], FP32R, name="g_rs1")
            g_rs2 = dpool.tile([P, D], FP32R, name="g_rs2")
            t_in = dpool.tile([D, D], FP32, name="t_in")
            t_rs1 = dpool.tile([P, D], FP32, name="t_rs1")
            t_rs2 = dpool.tile([P, D], FP32, name="t_rs2")
            _ashr = "Local" if mock_coll else "Shared"
            mx_in1 = dpool.tile([P, dT // 2], FP32, name="mx_in1")
            mx_out1 = dpool.tile([NC, P, dT // 2], FP32, name="mx_out1",
                                 addr_space=_ashr)
            mx_in2 = dpool.tile([P, dT // 2], FP32, name="mx_in2")
            mx_out2 = dpool.tile([NC, P, dT // 2], FP32, name="mx_out2",
                                 addr_space=_ashr)
            sums_in = dpool.tile([P, dT], FP16, name="sums_in")
            sums_out = dpool.tile([NC, P, dT], FP16, name="sums_out",
                                  addr_space=_ashr)
            # P^T chunks (fp8): a = j-rows 0:128, b = rows 128:256
            pa_in = dpool.tile([P, D], FP8, name="pa_in")
            pb_in = dpool.tile([P, D], FP8, name="pb_in")
            pa_out = dpool.tile([NC, P, D], FP8, name="pa_out",
                                addr_space=_ashr)
            pb_out = dpool.tile([NC, P, D], FP8, name="pb_out",
                                addr_space=_ashr)

            def coll(kind, op, in_ap, out_ap):
                if mock_coll:
                    return None
                return nc.gpsimd.collective_compute(
                    kind, op, replica_groups=[list(range(NC))],
                    ins=[in_ap], outs=[out_ap])

            # persistent pools (live across phases); xn on top of the
            # stack so it can release before phase A (SBUF headroom)
            v_pool = tc.alloc_tile_pool(name="vsb", bufs=1)
            xn_pool = tc.alloc_tile_pool(name="xn", bufs=1)
            xn_released = [False]
            # allocate xn tiles now, but load them after the G loop so the
            # 4.2MB doesn't delay g_in's drain on the serialized DMA engine
            xn_sb = [xn_pool.tile([P, NL], FP16, name=f"xn{k}")
                     for k in range(dT)]
            v_sb = [v_pool.tile([P, NL], FP16, name=f"v{iv}")
                    for iv in range(dT)]

            v_sweeps_done = [0]

            def emit_v_sweeps(n_sweeps):
                """V = W_k @ X, in sweeps of 4 iv-tiles each (4 total)."""
                s0 = v_sweeps_done[0]
                s1 = min(s0 + n_sweeps, 4)
                v_sweeps_done[0] = s1
                if s1 <= s0:
                    return
                with tc.tile_pool(name=f"wk{s0}", bufs=4) as wk_pool, \
                     tc.tile_pool(name=f"vps{s0}", bufs=8,
                                  space="PSUM") as vps_pool:
                    for sw in range(s0, s1):
                        pss = {}
                        for j in range(4):
                            for nb in range(NB):
                                pss[(j, nb)] = vps_pool.tile(
                                    [P, NBS], FP32, name="v_ps", tag="v_ps")
                        for k in range(dT):
                            wt = wk_pool.tile([P, 4 * P], FP16, name="wk_t",
                                              tag="wk_t")
                            nc.sync.dma_start(out=wt, in_=wkt_g[sw, k])
                            for j in range(4):
                                for nb in range(NB):
                                    ns = slice(nb * NBS, (nb + 1) * NBS)
                                    nc.tensor.matmul(
                                        pss[(j, nb)],
                                        wt[:, j * P:(j + 1) * P],
                                        xn_sb[k][:, ns],
                                        start=(k == 0), stop=(k == dT - 1))
                        for j in range(4):
                            for nb in range(NB):
                                ns = slice(nb * NBS, (nb + 1) * NBS)
                                nc.vector.tensor_copy(
                                    out=v_sb[sw * 4 + j][:, ns],
                                    in_=pss[(j, nb)])

            # ---------------- Phase G: G_c = X_c X_c^T (fp32r) --------------
            with tc.tile_pool(name="xt", bufs=1) as xt_pool:
                xt = [xt_pool.tile([P, D], FP32R, name=f"xt{n}")
                      for n in range(nT)]
                # column-chunked loads so the first (m, kb=0) group can
                # start after ~2MB instead of the full 8MB
                for q in range(KB):
                    cs = slice(q * CB, (q + 1) * CB)
                    for n in range(nT):
                        nc.sync.dma_start(out=xt[n][:, cs],
                                          in_=xt_f32[n * P:(n + 1) * P, cs])

                with tc.tile_pool(name="gstg", bufs=3) as gstg_pool, \
                     tc.tile_pool(name="gps", bufs=4,
                                  space="PSUM") as gps_pool:
                    for m in range(dT):
                        ms = slice(m * P, (m + 1) * P)
                        stg = gstg_pool.tile([P, D], FP32R, name="g_stg",
                                             tag="g_stg")
                        for kb in range(KB):
                            ks = slice(kb * CB, (kb + 1) * CB)
                            ps = gps_pool.tile([P, CB], FP32, name="g_ps",
                                               tag="g_ps")
                            for n in range(nT):
                                nc.tensor.matmul(ps, xt[n][:, ms],
                                                 xt[n][:, ks],
                                                 start=(n == 0),
                                                 stop=(n == nT - 1))
                            nc.vector.tensor_copy(out=stg[:, ks], in_=ps)
                        nc.sync.dma_start(out=g_in[ms, :], in_=stg)
                        # RS the first half as soon as its rows are written
                        if m == dT // 2 - 1 and stop_after != "g":
                            coll("ReduceScatter", mybir.AluOpType.add,
                                 g_in[0:D // 2, :].opt(), g_rs1.opt())

            if stop_after != "g":
                coll("ReduceScatter", mybir.AluOpType.add,
                     g_in[D // 2:D, :].opt(), g_rs2.opt())

            for k in range(dT):
                nc.sync.dma_start(out=xn_sb[k],
                                  in_=xn_f16[k * P:(k + 1) * P, :])
            emit_v_sweeps(v_slices[0])

            if stop_after not in ("g", "rs1"):
                # ------------- Phase T: T^T partials over local k ------------
                with tc.tile_pool(name="grs", bufs=1) as grs_pool, \
                     tc.tile_pool(name="wv", bufs=1) as wv_pool, \
                     tc.tile_pool(name="tstg", bufs=3) as tstg_pool, \
                     tc.tile_pool(name="tps", bufs=4, space="PSUM") as tps_pool:
                    grs = []
                    wv = []
                    for k, gsrc in ((0, g_rs1), (1, g_rs2)):
                        gt = grs_pool.tile([P, D], FP32R, name=f"grs{k}")
                        nc.sync.dma_start(out=gt, in_=gsrc[:, :])
                        grs.append(gt)
                        wt = wv_pool.tile([P, D], FP32R, name=f"wv{k}")
                        nc.sync.dma_start(out=wt,
                                          in_=wvts[k * P:(k + 1) * P, :])
                        wv.append(wt)
                    for j in range(dT):
                        js = slice(j * P, (j + 1) * P)
                        stg = tstg_pool.tile([P, D], FP32, name="t_stg",
                                             tag="t_stg")
                        for mb in range(KB):
                            mbs = slice(mb * CB, (mb + 1) * CB)
                            ps = tps_pool.tile([P, CB], FP32, name="t_ps",
                                               tag="t_ps")
                            for k in range(JT):
                                nc.tensor.matmul(ps, wv[k][:, js],
                                                 grs[k][:, mbs],
                                                 start=(k == 0),
                                                 stop=(k == JT - 1))
                            nc.vector.tensor_copy(out=stg[:, mbs], in_=ps)
                        nc.sync.dma_start(out=t_in[js, :], in_=stg)
                        if j == dT // 2 - 1 and stop_after != "t":
                            coll("ReduceScatter", mybir.AluOpType.add,
                                 t_in[0:D // 2, :].opt(), t_rs1.opt())

                if stop_after != "t":
                    coll("ReduceScatter", mybir.AluOpType.add,
                         t_in[D // 2:D, :].opt(), t_rs2.opt())

                emit_v_sweeps(v_slices[1])
                if v_sweeps_done[0] >= 4:
                    xn_pool.release()
                    xn_released[0] = True

            if stop_after not in ("g", "rs1", "t", "rs2"):
                # ------- Phase S: S[:, j_c] = W_q @ T, softmax, AG(P^T) ------
                with tc.tile_pool(name="trs", bufs=1) as trs_pool, \
                     tc.tile_pool(name="tmj", bufs=1) as tmj_pool, \
                     tc.tile_pool(name="idt", bufs=1) as idt_pool, \
                     tc.tile_pool(name="wq", bufs=12) as wq_pool, \
                     tc.tile_pool(name="ssb", bufs=1) as s_pool, \
                     tc.tile_pool(name="stat", bufs=1) as stat_pool, \
                     tc.tile_pool(name="ptl", bufs=1) as ptl_pool, \
                     tc.tile_pool(name="mps", bufs=2, space="PSUM") as mps_pool, \
                     tc.tile_pool(name="ptps", bufs=2, space="PSUM") as ptps_pool, \
                     tc.tile_pool(name="sps", bufs=4, space="PSUM") as sps_pool:
                    trs = []
                    for jt, tsrc in ((0, t_rs1), (1, t_rs2)):
                        t = trs_pool.tile([P, D], FP32, name=f"trs{jt}")
                        nc.sync.dma_start(out=t, in_=tsrc[:, :])
                        trs.append(t)
                    identR = idt_pool.tile([P, P], FP32, name="identR")
                    make_identity(nc, identR)
                    identH = idt_pool.tile([P, P], FP16, name="identH")
                    make_identity(nc, identH)
                    # transpose T^T[j_c, m] -> T[m, j_c] tiles [128, 256]
                    tmj = [tmj_pool.tile([P, JS], FP32R, name=f"tmj{mt}")
                           for mt in range(dT)]
                    for mt in range(dT):
                        mts = slice(mt * P, (mt + 1) * P)
                        for jt in range(JT):
                            mp = mps_pool.tile([P, P], FP32, name="m_ps",
                                               tag="m_ps")
                            nc.tensor.transpose(mp, trs[jt][:, mts], identR)
                            nc.vector.tensor_copy(
                                out=tmj[mt][:, jt * P:(jt + 1) * P], in_=mp)

                    mx = stat_pool.tile([P, dT], FP32, name="mx")
                    gmx = stat_pool.tile([P, dT], FP32, name="gmx")
                    negm = stat_pool.tile([P, dT], FP32, name="negm")
                    lsum = stat_pool.tile([P, dT], FP32, name="lsum")

                    s_sb = []
                    for i in range(dT):
                        wq_i = wq_pool.tile([P, dT, P], FP32R, name="wq_i",
                                            tag="wq_i")
                        nc.sync.dma_start(out=wq_i, in_=wqt_b[i])
                        ps = sps_pool.tile([P, JS], FP32, name="s_ps",
                                           tag="s_ps")
                        for mt in range(dT):
                            nc.tensor.matmul(ps, wq_i[:, mt, :],
                                             tmj[mt],
                                             start=(mt == 0),
                                             stop=(mt == dT - 1))
                        st = s_pool.tile([P, JS], FP32, name=f"s{i}")
                        nc.scalar.copy(st, ps)
                        nc.vector.reduce_max(mx[:, i:i + 1], st,
                                             axis=mybir.AxisListType.X)
                        s_sb.append(st)
                        # AG the first half's local maxes while the second
                        # half of S still computes (hides one latency)
                        if i == dT // 2 - 1:
                            nc.sync.dma_start(out=mx_in1,
                                              in_=mx[:, :dT // 2])
                            if stop_after != "s":
                                coll("AllGather", mybir.AluOpType.bypass,
                                     mx_in1, mx_out1)
                    nc.sync.dma_start(out=mx_in2, in_=mx[:, dT // 2:])
                    if stop_after != "s":
                        coll("AllGather", mybir.AluOpType.bypass,
                             mx_in2, mx_out2)
                    mxg = stat_pool.tile([P, NC, dT], FP32, name="mxg")
                    pta = ptl_pool.tile([P, D], FP16, name="pta")
                    ptb = ptl_pool.tile([P, D], FP16, name="ptb")
                    pta8 = ptl_pool.tile([P, D], FP8, name="pta8")
                    ptb8 = ptl_pool.tile([P, D], FP8, name="ptb8")
                    H = dT // 2
                    for half, mxo in ((0, mx_out1), (1, mx_out2)):
                        hs = slice(half * H, (half + 1) * H)
                        nc.sync.dma_start(
                            out=mxg[:, :, hs],
                            in_=mxo.rearrange("r p i -> p r i"))
                        nc.vector.reduce_max(
                            gmx[:, hs],
                            mxg[:, :, hs].rearrange("p r i -> p i r"),
                            axis=mybir.AxisListType.X)
                        nc.scalar.mul(negm[:, hs], gmx[:, hs], -1.0)
                        for i in range(half * H, (half + 1) * H):
                            pt = ptl_pool.tile([P, JS], FP16, name=f"pe{i}",
                                               tag="pe")
                            nc.scalar.activation(pt, s_sb[i], AF.Exp,
                                                 bias=negm[:, i:i + 1],
                                                 scale=1.0,
                                                 accum_out=lsum[:, i:i + 1])
                            isl = slice(i * P, (i + 1) * P)
                            for jt, dst in ((0, pta), (1, ptb)):
                                mp = ptps_pool.tile([P, P], FP16,
                                                    name="pt_ps",
                                                    tag="pt_ps")
                                nc.tensor.transpose(
                                    mp, pt[:, jt * P:(jt + 1) * P], identH)
                                nc.vector.tensor_copy(out=dst[:, isl],
                                                      in_=mp)
                        hcs = slice(half * D // 2, (half + 1) * D // 2)
                        nc.vector.tensor_copy(out=pta8[:, hcs],
                                              in_=pta[:, hcs])
                        nc.vector.tensor_copy(out=ptb8[:, hcs],
                                              in_=ptb[:, hcs])
                    lsum16 = stat_pool.tile([P, dT], FP16, name="lsum16")
                    nc.vector.tensor_copy(out=lsum16, in_=lsum)
                    nc.sync.dma_start(out=sums_in, in_=lsum16)
                    nc.sync.dma_start(out=pa_in[:, :], in_=pta8)
                    nc.sync.dma_start(out=pb_in[:, :], in_=ptb8)

                    if stop_after != "s":
                        cc_pa = coll("AllGather", mybir.AluOpType.bypass,
                                     pa_in, pa_out)
                        cc_pb = coll("AllGather", mybir.AluOpType.bypass,
                                     pb_in, pb_out)
                        cc_sums = coll("AllGather", mybir.AluOpType.bypass,
                                       sums_in, sums_out)
                        if cc_pa is not None:
                            # keep the serial collective engine on the
                            # critical order P-a, P-b, sums
                            tile.add_dep_helper(
                                cc_sums.ins, cc_pb.ins, sync=False,
                                reason="sums AG after P chunk AGs")

                emit_v_sweeps(4)  # any remaining V fills the AG window
                if not xn_released[0]:
                    xn_pool.release()
                    xn_released[0] = True

            if stop_after in (None, "all"):
                # ------- Phase A: A = P V in two passes (a overlaps AG-b) ----
                with tc.tile_pool(name="ptg", bufs=1) as ptg_pool, \
                     tc.tile_pool(name="ptg8", bufs=1) as ptg8_pool, \
                     tc.tile_pool(name="sums", bufs=1) as sums_pool, \
                     tc.tile_pool(name="aacc", bufs=1) as aacc_pool, \
                     tc.tile_pool(name="asb", bufs=3) as a_pool:

                    # global row sums from the gathered local sums
                    sums_g = sums_pool.tile([P, NC, dT], FP16, name="sums_g")
                    nc.sync.dma_start(
                        out=sums_g, in_=sums_out.rearrange("r p i -> p r i"))
                    ssum = sums_pool.tile([P, dT], FP32, name="ssum")
                    recip = sums_pool.tile([P, dT], FP32, name="recip")
                    nc.vector.reduce_sum(
                        ssum, sums_g.rearrange("p r i -> p i r"),
                        axis=mybir.AxisListType.X)
                    nc.vector.reciprocal(recip, ssum)

                    pta16 = [ptg_pool.tile([P, D], FP16, name=f"ptga{r}")
                             for r in range(NC)]
                    ptb16 = [ptg_pool.tile([P, D], FP16, name=f"ptgb{r}")
                             for r in range(NC)]
                    pt8a = ptg8_pool.tile([P, NC, D], FP8, name="pt8a")
                    pt8b = ptg8_pool.tile([P, NC, D], FP8, name="pt8b")

                    nc.sync.dma_start(out=pt8a,
                                      in_=pa_out.rearrange("r p c -> p r c"))
                    for r in range(NC):
                        nc.vector.tensor_copy(out=pta16[r], in_=pt8a[:, r, :])

                    # pass a: accumulate even jt tiles, drain to fp16 SBUF
                    asb_a = [aacc_pool.tile([P, NL], FP16, name=f"aa{i}")
                             for i in range(dT)]
                    apsa_pool = tc.alloc_tile_pool(name="apsA", bufs=4,
                                                   space="PSUM")
                    last_a = None
                    for i in range(dT):
                        isl = slice(i * P, (i + 1) * P)
                        apa = apsa_pool.tile([P, NL], FP32, name="a_psA",
                                             tag="a_psA")
                        for rr in range(NC):
                            for nb in range(NB):
                                ns = slice(nb * NBS, (nb + 1) * NBS)
                                last_a = nc.tensor.matmul(
                                    apa[:, ns], pta16[rr][:, isl],
                                    v_sb[rr][:, ns],
                                    start=(rr == 0), stop=(rr == NC - 1))
                        nc.vector.tensor_copy(out=asb_a[i], in_=apa)
                    apsa_pool.release()

                    nc.sync.dma_start(out=pt8b,
                                      in_=pb_out.rearrange("r p c -> p r c"))
                    for r in range(NC):
                        nc.vector.tensor_copy(out=ptb16[r], in_=pt8b[:, r, :])

                    # pass b: odd jt tiles + combine + scale
                    identA = sums_pool.tile([P, P], FP16, name="identA")
                    make_identity(nc, identA)
                    apsb_pool = tc.alloc_tile_pool(name="apsB", bufs=4,
                                                   space="PSUM")
                    for i in range(dT):
                        isl = slice(i * P, (i + 1) * P)
                        apb = apsb_pool.tile([P, NL], FP32, name="a_psB",
                                             tag="a_psB")
                        for rr in range(NC):
                            for nb in range(NB):
                                ns = slice(nb * NBS, (nb + 1) * NBS)
                                mm = nc.tensor.matmul(
                                    apb[:, ns], ptb16[rr][:, isl],
                                    v_sb[NC + rr][:, ns],
                                    start=(rr == 0), stop=False)
                                if i == 0 and rr == 0 and nb == 0 \
                                        and last_a is not None:
                                    # keep the in-order PE stream from
                                    # blocking on AG-Pb data while pass-a
                                    # work is still pending behind it
                                    tile.add_dep_helper(
                                        mm.ins, last_a.ins, sync=False,
                                        reason="pass-b after all of pass-a")
                        # fold the pass-a partial in via an identity matmul
                        for nb in range(NB):
                            ns = slice(nb * NBS, (nb + 1) * NBS)
                            nc.tensor.matmul(apb[:, ns], identA,
                                             asb_a[i][:, ns],
                                             start=False, stop=(nb == NB - 1))
                        asb = a_pool.tile([P, NL], FP32, name="a_sb",
                                          tag="a_sb")
                        nc.vector.tensor_scalar_mul(asb, apb,
                                                    recip[:, i:i + 1])
                        nc.sync.dma_start(out=a_out[i * P:(i + 1) * P, :],
                                          in_=asb)
                    apsb_pool.release()

            if not xn_released[0]:
                xn_pool.release()
            v_pool.release()

    nc.compile()
    return nc


def prepare_inputs(X_t, W_q, W_k, W_v, NC=NCORES):
    """Host-side sharding.  Returns in_maps for SPMD."""
    D, N = X_t.shape
    NL = N // NC
    KS = D // NC
    P_ = 128
    dT = D // P_
    sc = np.float32(1.0) / np.sqrt(np.float32(D))

    X_t = np.asarray(X_t, np.float32)
    wvt = (np.asarray(W_v, np.float32).T * sc)
    wqt = np.asarray(W_q, np.float32).T
    # [i, p, m, q] = Wq^T[m*128+p, i*128+q]  (partition axis p first per i)
    wqt_b = np.ascontiguousarray(
        wqt.reshape(dT, P_, dT, P_).transpose(2, 1, 0, 3))
    wkt = np.ascontiguousarray(np.asarray(W_k, np.float32).T
                               .astype(np.float16))
    wkt_g = np.ascontiguousarray(
        wkt.reshape(dT, P_, dT // 4, 4 * P_).transpose(2, 0, 1, 3))

    in_maps = []
    for c in range(NC):
        xc = np.ascontiguousarray(X_t[:, c * NL:(c + 1) * NL])
        in_maps.append({
            "xt_f32": np.ascontiguousarray(xc.T),
            "xn_f16": np.ascontiguousarray(xc.astype(np.float16)),
            "wkt_g": wkt_g,
            "wvts": np.ascontiguousarray(np.concatenate(
                [wvt[c * P_:(c + 1) * P_, :],
                 wvt[D // 2 + c * P_:D // 2 + (c + 1) * P_, :]], axis=0)),
            "wqt_b": wqt_b,
        })
    return in_maps


_CACHED_NC = None


def _get_nc():
    global _CACHED_NC
    if _CACHED_NC is None:
        _CACHED_NC = build()
    return _CACHED_NC


def run(X_t, W_q, W_k, W_v, trace=False):
    from concourse.bass_utils import run_bass_kernel_spmd
    nc = _get_nc()
    in_maps = prepare_inputs(X_t, W_q, W_k, W_v)
    res = run_bass_kernel_spmd(nc, in_maps, core_ids=list(range(NCORES)),
                               trace=trace)
    A = np.concatenate([res.results[c]["a_out"] for c in range(NCORES)],
                       axis=1)
    return A, res


def kernel(X_t, W_q, W_k, W_v):
    X_t = np.asarray(X_t)
    W_q = np.asarray(W_q)
    W_k = np.asarray(W_k)
    W_v = np.asarray(W_v)
    A, _ = run(X_t, W_q, W_k, W_v, trace=False)
    return A.astype(np.float32)


# revision 59
# speedup vs baseline: 1.3985x; 1.3985x over previous
"""Distributed AttentionAutoEncoder kernel for 8 TRN2 NeuronCores (Bass/Tile).

Reference computation (fp32):
    Q = W_q @ X ; K = W_v @ X ; V = W_k @ X          (d=2048, n=8192)
    S = (Q @ K.T) / sqrt(d) ; Z = softmax(S, -1) ; A = Z @ V

Reformulation: S = W_q G W_v'^T with G = X X^T (contracting n) and
W_v' = W_v^T/sqrt(d).  n is sharded 1024/core; the cross-core reduction
is pushed through the chain as ReduceScatters (only the scattered
output is charged by the interconnect) instead of AllReducing G:

  G_c = X_c X_c^T            full d x d, single-pass float32r (fp22)
  RS1 (2 halves, first issued mid-G): G rows {128c, 1024+128c} summed
  T^T[j,:] = sum_{k local} Wv'[k,j] G[k,:]      (fp32r)
  RS2 (2 halves, first issued mid-T): T^T rows j_c = {128c, 1024+128c}
  S[:, j_c] = W_q @ T[:, j_c]                   (fp32r, after 32 PE
        transposes of the T^T shard)
  softmax: local row maxes AllGathered in 2 halves (first hides under
        the S loop), reduced locally; exp to fp16 then PE-transposed
        and converted to fp8
  AG: P^T in two fp8 chunks (j-tiles c and 8+c) + a tiny AllGather of
        the local exp-sums, ordered P-a, P-b, sums so the A-loop's
        first half overlaps chunk b on the serial collective engine
  A = P V: gathered P^T tiles (fp8 -> fp16) are used directly as
        stationary operands - no transposes in the tail; the pass-a
        partial is drained to fp16 SBUF and folded back into pass-b's
        PSUM with an identity matmul; 1/rowsum is applied in the final
        per-row scale.

V = W_k X_c is data-parallel fp16, emitted in sweeps between the
phases so the in-order PE fills the collective latency windows.
G/T/S matmuls are single-pass float32r (1 cyc/row at moving >= 256);
V/A are fp16.  End-to-end fro error ~1.8e-3 on hardware vs the 2e-2
gate (the fp22 S chain dominates; fp8 P costs nothing extra).
"""

import numpy as np

import concourse.bacc as bacc
import concourse.mybir as mybir
import concourse.tile as tile
from concourse.masks import make_identity

P = 128
FP16 = mybir.dt.float16
FP32 = mybir.dt.float32
FP32R = mybir.dt.float32r
FP8 = mybir.dt.float8e4
AF = mybir.ActivationFunctionType

D_FULL = 2048
N_FULL = 8192
NCORES = 8


def build(D=D_FULL, NL=N_FULL // NCORES, NC=NCORES, stop_after=None,
          mock_coll=False, v_slices=(2, 2, 0)):
    """Build the SPMD Bass program (identical on every core)."""
    KS = D // NC          # k-rows of reduced G / j-rows of T^T per core
    JS = D // NC
    nT = NL // P          # 8
    dT = D // P           # 16
    CB = 512              # fp32 moving max
    KB = D // CB          # 4
    NBS = 512
    NB = NL // NBS        # 2
    JT = KS // P          # 2 j-tiles per core

    nc = bacc.Bacc("TRN2", target_bir_lowering=False, debug=False,
                   num_devices=NC)

    # ------------- I/O -------------
    xt_f32 = nc.dram_tensor("xt_f32", [NL, D], FP32R, kind="ExternalInput")
    xn_f16 = nc.dram_tensor("xn_f16", [D, NL], FP16, kind="ExternalInput")
    wkt_g = nc.dram_tensor("wkt_g", [dT // 4, dT, P, 4 * P], FP16,
                           kind="ExternalInput")
    wvts = nc.dram_tensor("wvts", [KS, D], FP32R, kind="ExternalInput")
    wqt_b = nc.dram_tensor("wqt_b", [dT, P, dT, P], FP32R,
                           kind="ExternalInput")
    a_out = nc.dram_tensor("a_out", [D, NL], FP32, kind="ExternalOutput")

    with tile.TileContext(nc) as tc:
        with tc.tile_pool(name="dram", bufs=1, space="DRAM") as dpool:
            g_in = dpool.tile([D, D], FP32R, name="g_in")
            g_rs1 = dpool.tile([P, D], FP32R, name="g_rs1")
            g_rs2 = dpool.tile([P, D], FP32R, name="g_rs2")
            t_in = dpool.tile([D, D], FP32, name="t_in")
            t_rs1 = dpool.tile([P, D], FP32, name="t_rs1")
            t_rs2 = dpool.tile([P, D], FP32, name="t_rs2")
            _ashr = "Local" if mock_coll else "Shared"
            mx_in1 = dpool.tile([P, dT // 2], FP32, name="mx_in1")
            mx_out1 = dpool.tile([NC, P, dT // 2], FP32, name="mx_out1",
                                 addr_space=_ashr)
            mx_in2 = dpool.tile([P, dT // 2], FP32, name="mx_in2")
            mx_out2 = dpool.tile([NC, P, dT // 2], FP32, name="mx_out2",
                                 addr_space=_ashr)
            sums_in = dpool.tile([P, dT], FP16, name="sums_in")
            sums_out = dpool.tile([NC, P, dT], FP16, name="sums_out",
                                  addr_space=_ashr)
            # P^T chunks (fp8): a = j-rows 0:128, b = rows 128:256
            pa_in = dpool.tile([P, D], FP8, name="pa_in")
            pb_in = dpool.tile([P, D], FP8, name="pb_in")
            pa_out = dpool.tile([NC, P, D], FP8, name="pa_out",
                                addr_space=_ashr)
            pb_out = dpool.tile([NC, P, D], FP8, name="pb_out",
                                addr_space=_ashr)

            def coll(kind, op, in_ap, out_ap):
                if mock_coll:
                    return None
                return nc.gpsimd.collective_compute(
                    kind, op, replica_groups=[list(range(NC))],
                    ins=[in_ap], outs=[out_ap])

            # persistent pools (live across phases); xn on top of the
            # stack so it can release before phase A (SBUF headroom)
            v_pool = tc.alloc_tile_pool(name="vsb", bufs=1)
            xn_pool = tc.alloc_tile_pool(name="xn", bufs=1)
            xn_released = [False]
            # allocate xn tiles now, but load them after the G loop so the
            # 4.2MB doesn't delay g_in's drain on the serialized DMA engine
            xn_sb = [xn_pool.tile([P, NL], FP16, name=f"xn{k}")
                     for k in range(dT)]
            v_sb = [v_pool.tile([P, NL], FP16, name=f"v{iv}")
                    for iv in range(dT)]

            v_sweeps_done = [0]

            def emit_v_sweeps(n_sweeps):
                """V = W_k @ X, in sweeps of 4 iv-tiles each (4 total)."""
                s0 = v_sweeps_done[0]
                s1 = min(s0 + n_sweeps, 4)
                v_sweeps_done[0] = s1
                if s1 <= s0:
                    return
                with tc.tile_pool(name=f"wk{s0}", bufs=4) as wk_pool, \
                     tc.tile_pool(name=f"vps{s0}", bufs=8,
                                  space="PSUM") as vps_pool:
                    for sw in range(s0, s1):
                        pss = {}
                        for j in range(4):
                            for nb in range(NB):
                                pss[(j, nb)] = vps_pool.tile(
                                    [P, NBS], FP32, name="v_ps", tag="v_ps")
                        for k in range(dT):
                            wt = wk_pool.tile([P, 4 * P], FP16, name="wk_t",
                                              tag="wk_t")
                            nc.sync.dma_start(out=wt, in_=wkt_g[sw, k])
                            for j in range(4):
                                for nb in range(NB):
                                    ns = slice(nb * NBS, (nb + 1) * NBS)
                                    nc.tensor.matmul(
                                        pss[(j, nb)],
                                        wt[:, j * P:(j + 1) * P],
                                        xn_sb[k][:, ns],
                                        start=(k == 0), stop=(k == dT - 1))
                        for j in range(4):
                            for nb in range(NB):
                                ns = slice(nb * NBS, (nb + 1) * NBS)
                                nc.vector.tensor_copy(
                                    out=v_sb[sw * 4 + j][:, ns],
                                    in_=pss[(j, nb)])

            # ---------------- Phase G: G_c = X_c X_c^T (fp32r) --------------
            with tc.tile_pool(name="xt", bufs=1) as xt_pool:
                xt = [xt_pool.tile([P, D], FP32R, name=f"xt{n}")
                      for n in range(nT)]
                # column-chunked loads so the first (m, kb=0) group can
                # start after ~2MB instead of the full 8MB
                for q in range(KB):
                    cs = slice(q * CB, (q + 1) * CB)
                    for n in range(nT):
                        nc.sync.dma_start(out=xt[n][:, cs],
                                          in_=xt_f32[n * P:(n + 1) * P, cs])

                with tc.tile_pool(name="gstg", bufs=3) as gstg_pool, \
                     tc.tile_pool(name="gps", bufs=4,
                                  space="PSUM") as gps_pool:
                    for m in range(dT):
                        ms = slice(m * P, (m + 1) * P)
                        stg = gstg_pool.tile([P, D], FP32R, name="g_stg",
                                             tag="g_stg")
                        for kb in range(KB):
                            ks = slice(kb * CB, (kb + 1) * CB)
                            ps = gps_pool.tile([P, CB], FP32, name="g_ps",
                                               tag="g_ps")
                            for n in range(nT):
                                nc.tensor.matmul(ps, xt[n][:, ms],
                                                 xt[n][:, ks],
                                                 start=(n == 0),
                                                 stop=(n == nT - 1))
                            nc.vector.tensor_copy(out=stg[:, ks], in_=ps)
                        nc.sync.dma_start(out=g_in[ms, :], in_=stg)
                        # RS the first half as soon as its rows are written
                        if m == dT // 2 - 1 and stop_after != "g":
                            coll("ReduceScatter", mybir.AluOpType.add,
                                 g_in[0:D // 2, :].opt(), g_rs1.opt())

            if stop_after != "g":
                coll("ReduceScatter", mybir.AluOpType.add,
                     g_in[D // 2:D, :].opt(), g_rs2.opt())

            for k in range(dT):
                nc.sync.dma_start(out=xn_sb[k],
                                  in_=xn_f16[k * P:(k + 1) * P, :])
            emit_v_sweeps(v_slices[0])

            if stop_after not in ("g", "rs1"):
                # ------------- Phase T: T^T partials over local k ------------
                with tc.tile_pool(name="grs", bufs=1) as grs_pool, \
                     tc.tile_pool(name="wv", bufs=1) as wv_pool, \
                     tc.tile_pool(name="tstg", bufs=3) as tstg_pool, \
                     tc.tile_pool(name="tps", bufs=4, space="PSUM") as tps_pool:
                    grs = []
                    wv = []
                    for k, gsrc in ((0, g_rs1), (1, g_rs2)):
                        gt = grs_pool.tile([P, D], FP32R, name=f"grs{k}")
                        nc.sync.dma_start(out=gt, in_=gsrc[:, :])
                        grs.append(gt)
                        wt = wv_pool.tile([P, D], FP32R, name=f"wv{k}")
                        nc.sync.dma_start(out=wt,
                                          in_=wvts[k * P:(k + 1) * P, :])
                        wv.append(wt)
                    for j in range(dT):
                        js = slice(j * P, (j + 1) * P)
                        stg = tstg_pool.tile([P, D], FP32, name="t_stg",
                                             tag="t_stg")
                        for mb in range(KB):
                            mbs = slice(mb * CB, (mb + 1) * CB)
                            ps = tps_pool.tile([P, CB], FP32, name="t_ps",
                                               tag="t_ps")
                            for k in range(JT):
                                nc.tensor.matmul(ps, wv[k][:, js],
                                                 grs[k][:, mbs],
                                                 start=(k == 0),
                                                 stop=(k == JT - 1))
                            nc.vector.tensor_copy(out=stg[:, mbs], in_=ps)
                        nc.sync.dma_start(out=t_in[js, :], in_=stg)
                        if j == dT // 2 - 1 and stop_after != "t":
                            coll("ReduceScatter", mybir.AluOpType.add,
                                 t_in[0:D // 2, :].opt(), t_rs1.opt())

                if stop_after != "t":
                    coll("ReduceScatter", mybir.AluOpType.add,
                         t_in[D // 2:D, :].opt(), t_rs2.opt())

                emit_v_sweeps(v_slices[1])
                if v_sweeps_done[0] >= 4:
                    xn_pool.release()
                    xn_released[0] = True

            if stop_after not in ("g", "rs1", "t", "rs2"):
                # ------- Phase S: S[:, j_c] = W_q @ T, softmax, AG(P^T) ------
                with tc.tile_pool(name="trs", bufs=1) as trs_pool, \
                     tc.tile_pool(name="tmj", bufs=1) as tmj_pool, \
                     tc.tile_pool(name="idt", bufs=1) as idt_pool, \
                     tc.tile_pool(name="wq", bufs=12) as wq_pool, \
                     tc.tile_pool(name="ssb", bufs=1) as s_pool, \
                     tc.tile_pool(name="stat", bufs=1) as stat_pool, \
                     tc.tile_pool(name="ptl", bufs=1) as ptl_pool, \
                     tc.tile_pool(name="mps", bufs=2, space="PSUM") as mps_pool, \
                     tc.tile_pool(name="ptps", bufs=2, space="PSUM") as ptps_pool, \
                     tc.tile_pool(name="sps", bufs=4, space="PSUM") as sps_pool:
                    trs = []
                    for jt, tsrc in ((0, t_rs1), (1, t_rs2)):
                        t = trs_pool.tile([P, D], FP32, name=f"trs{jt}")
                        nc.sync.dma_start(out=t, in_=tsrc[:, :])
                        trs.append(t)
                    identR = idt_pool.tile([P, P], FP32, name="identR")
                    make_identity(nc, identR)
                    identH = idt_pool.tile([P, P], FP16, name="identH")
                    make_identity(nc, identH)
                    # transpose T^T[j_c, m] -> T[m, j_c] tiles [128, 256]
                    tmj = [tmj_pool.tile([P, JS], FP32R, name=f"tmj{mt}")
                           for mt in range(dT)]
                    for mt in range(dT):
                        mts = slice(mt * P, (mt + 1) * P)
                        for jt in range(JT):
                            mp = mps_pool.tile([P, P], FP32, name="m_ps",
                                               tag="m_ps")
                            nc.tensor.transpose(mp, trs[jt][:, mts], identR)
                            nc.vector.tensor_copy(
                                out=tmj[mt][:, jt * P:(jt + 1) * P], in_=mp)

                    mx = stat_pool.tile([P, dT], FP32, name="mx")
                    gmx = stat_pool.tile([P, dT], FP32, name="gmx")
                    negm = stat_pool.tile([P, dT], FP32, name="negm")
                    lsum = stat_pool.tile([P, dT], FP32, name="lsum")

                    s_sb = []
                    for i in range(dT):
                        wq_i = wq_pool.tile([P, dT, P], FP32R, name="wq_i",
                                            tag="wq_i")
                        nc.sync.dma_start(out=wq_i, in_=wqt_b[i])
                        ps = sps_pool.tile([P, JS], FP32, name="s_ps",
                                           tag="s_ps")
                        for mt in range(dT):
                            nc.tensor.matmul(ps, wq_i[:, mt, :],
                                             tmj[mt],
                                             start=(mt == 0),
                                             stop=(mt == dT - 1))
                        st = s_pool.tile([P, JS], FP32, name=f"s{i}")
                        nc.scalar.copy(st, ps)
                        nc.vector.reduce_max(mx[:, i:i + 1], st,
                                             axis=mybir.AxisListType.X)
                        s_sb.append(st)
                        # AG the first half's local maxes while the second
                        # half of S still computes (hides one latency)
                        if i == dT // 2 - 1:
                            nc.sync.dma_start(out=mx_in1,
                                              in_=mx[:, :dT // 2])
                            if stop_after != "s":
                                coll("AllGather", mybir.AluOpType.bypass,
                                     mx_in1, mx_out1)
                    nc.sync.dma_start(out=mx_in2, in_=mx[:, dT // 2:])
                    if stop_after != "s":
                        coll("AllGather", mybir.AluOpType.bypass,
                             mx_in2, mx_out2)
                    mxg = stat_pool.tile([P, NC, dT], FP32, name="mxg")
                    pta = ptl_pool.tile([P, D], FP16, name="pta")
                    ptb = ptl_pool.tile([P, D], FP16, name="ptb")
                    pta8 = ptl_pool.tile([P, D], FP8, name="pta8")
                    ptb8 = ptl_pool.tile([P, D], FP8, name="ptb8")
                    H = dT // 2
                    for half, mxo in ((0, mx_out1), (1, mx_out2)):
                        hs = slice(half * H, (half + 1) * H)
                        nc.sync.dma_start(
                            out=mxg[:, :, hs],
                            in_=mxo.rearrange("r p i -> p r i"))
                        nc.vector.reduce_max(
                            gmx[:, hs],
                            mxg[:, :, hs].rearrange("p r i -> p i r"),
                            axis=mybir.AxisListType.X)
                        nc.scalar.mul(negm[:, hs], gmx[:, hs], -1.0)
                        for i in range(half * H, (half + 1) * H):
                            pt = ptl_pool.tile([P, JS], FP16, name=f"pe{i}",
                                               tag="pe")
                            nc.scalar.activation(pt, s_sb[i], AF.Exp,
                                                 bias=negm[:, i:i + 1],
                                                 scale=1.0,
                                                 accum_out=lsum[:, i:i + 1])
                            isl = slice(i * P, (i + 1) * P)
                            for jt, dst in ((0, pta), (1, ptb)):
                                mp = ptps_pool.tile([P, P], FP16,
                                                    name="pt_ps",
                                                    tag="pt_ps")
                                nc.tensor.transpose(
                                    mp, pt[:, jt * P:(jt + 1) * P], identH)
                                nc.vector.tensor_copy(out=dst[:, isl],
                                                      in_=mp)
                        hcs = slice(half * D // 2, (half + 1) * D // 2)
                        nc.vector.tensor_copy(out=pta8[:, hcs],
                                              in_=pta[:, hcs])
                        nc.vector.tensor_copy(out=ptb8[:, hcs],
                                              in_=ptb[:, hcs])
                    lsum16 = stat_pool.tile([P, dT], FP16, name="lsum16")
                    nc.vector.tensor_copy(out=lsum16, in_=lsum)
                    nc.sync.dma_start(out=sums_in, in_=lsum16)
                    nc.sync.dma_start(out=pa_in[:, :], in_=pta8)
                    nc.sync.dma_start(out=pb_in[:, :], in_=ptb8)

                    if stop_after != "s":
                        cc_pa = coll("AllGather", mybir.AluOpType.bypass,
                                     pa_in, pa_out)
                        cc_pb = coll("AllGather", mybir.AluOpType.bypass,
                                     pb_in, pb_out)
                        cc_sums = coll("AllGather", mybir.AluOpType.bypass,
                                       sums_in, sums_out)
                        if cc_pa is not None:
                            # keep the serial collective engine on the
                            # critical order P-a, P-b, sums
                            tile.add_dep_helper(
                                cc_sums.ins, cc_pb.ins, sync=False,
                                reason="sums AG after P chunk AGs")

                emit_v_sweeps(4)  # any remaining V fills the AG window
                if not xn_released[0]:
                    xn_pool.release()
                    xn_released[0] = True

            if stop_after in (None, "all"):
                # ------- Phase A: A = P V in two passes (a overlaps AG-b) ----
                with tc.tile_pool(name="ptg", bufs=1) as ptg_pool, \
                     tc.tile_pool(name="ptg8", bufs=1) as ptg8_pool, \
                     tc.tile_pool(name="sums", bufs=1) as sums_pool, \
                     tc.tile_pool(name="aacc", bufs=1) as aacc_pool, \
                     tc.tile_pool(name="asb", bufs=3) as a_pool:

                    # global row sums from the gathered local sums
                    sums_g = sums_pool.tile([P, NC, dT], FP16, name="sums_g")
                    nc.sync.dma_start(
                        out=sums_g, in_=sums_out.rearrange("r p i -> p r i"))
                    ssum = sums_pool.tile([P, dT], FP32, name="ssum")
                    recip = sums_pool.tile([P, dT], FP32, name="recip")
                    nc.vector.reduce_sum(
                        ssum, sums_g.rearrange("p r i -> p i r"),
                        axis=mybir.AxisListType.X)
                    nc.vector.reciprocal(recip, ssum)

                    pta16 = [ptg_pool.tile([P, D], FP16, name=f"ptga{r}")
                             for r in range(NC)]
                    ptb16 = [ptg_pool.tile([P, D], FP16, name=f"ptgb{r}")
                             for r in range(NC)]
                    pt8a = ptg8_pool.tile([P, NC, D], FP8, name="pt8a")
                    pt8b = ptg8_pool.tile([P, NC, D], FP8, name="pt8b")

                    nc.sync.dma_start(out=pt8a,
                                      in_=pa_out.rearrange("r p c -> p r c"))
                    for r in range(NC):
                        nc.vector.tensor_copy(out=pta16[r], in_=pt8a[:, r, :])

                    # pass a: accumulate even jt tiles, drain to fp16 SBUF
                    asb_a = [aacc_pool.tile([P, NL], FP16, name=f"aa{i}")
                             for i in range(dT)]
                    apsa_pool = tc.alloc_tile_pool(name="apsA", bufs=4,
                                                   space="PSUM")
                    last_a = None
                    for i in range(dT):
                        isl = slice(i * P, (i + 1) * P)
                        apa = apsa_pool.tile([P, NL], FP32, name="a_psA",
                                             tag="a_psA")
                        for rr in range(NC):
                            for nb in range(NB):
                                ns = slice(nb * NBS, (nb + 1) * NBS)
                                last_a = nc.tensor.matmul(
                                    apa[:, ns], pta16[rr][:, isl],
                                    v_sb[rr][:, ns],
                                    start=(rr == 0), stop=(rr == NC - 1))
                        nc.vector.tensor_copy(out=asb_a[i], in_=apa)
                    apsa_pool.release()

                    nc.sync.dma_start(out=pt8b,
                                      in_=pb_out.rearrange("r p c -> p r c"))
                    for r in range(NC):
                        nc.vector.tensor_copy(out=ptb16[r], in_=pt8b[:, r, :])

                    # pass b: odd jt tiles + combine + scale
                    identA = sums_pool.tile([P, P], FP16, name="identA")
                    make_identity(nc, identA)
                    apsb_pool = tc.alloc_tile_pool(name="apsB", bufs=4,
                                                   space="PSUM")
                    for i in range(dT):
                        isl = slice(i * P, (i + 1) * P)
                        apb = apsb_pool.tile([P, NL], FP32, name="a_psB",
                                             tag="a_psB")
                        for rr in range(NC):
                            for nb in range(NB):
                                ns = slice(nb * NBS, (nb + 1) * NBS)
                                mm = nc.tensor.matmul(
                                    apb[:, ns], ptb16[rr][:, isl],
                                    v_sb[NC + rr][:, ns],
                                    start=(rr == 0), stop=False)
                                if i == 0 and rr == 0 and nb == 0 \
                                        and last_a is not None:
                                    # keep the in-order PE stream from
                                    # blocking on AG-Pb data while pass-a
                                    # work is still pending behind it
                                    tile.add_dep_helper(
                                        mm.ins, last_a.ins, sync=False,
                                        reason="pass-b after all of pass-a")
                        # fold the pass-a partial in via an identity matmul
                        for nb in range(NB):
                            ns = slice(nb * NBS, (nb + 1) * NBS)
                            nc.tensor.matmul(apb[:, ns], identA,
                                             asb_a[i][:, ns],
                                             start=False, stop=(nb == NB - 1))
                        asb = a_pool.tile([P, NL], FP32, name="a_sb",
                                          tag="a_sb")
                        nc.vector.tensor_scalar_mul(asb, apb,
                                                    recip[:, i:i + 1])
                        nc.sync.dma_start(out=a_out[i * P:(i + 1) * P, :],
                                          in_=asb)
                    apsb_pool.release()

            if not xn_released[0]:
                xn_pool.release()
            v_pool.release()

    nc.compile()
    return nc


def prepare_inputs(X_t, W_q, W_k, W_v, NC=NCORES):
    """Host-side sharding.  Returns in_maps for SPMD."""
    D, N = X_t.shape
    NL = N // NC
    KS = D // NC
    P_ = 128
    dT = D // P_
    sc = np.float32(1.0) / np.sqrt(np.float32(D))

    X_t = np.asarray(X_t, np.float32)
    wvt = (np.asarray(W_v, np.float32).T * sc)
    wqt = np.asarray(W_q, np.float32).T
    # [i, p, m, q] = Wq^T[m*128+p, i*128+q]  (partition axis p first per i)
    wqt_b = np.ascontiguousarray(
        wqt.reshape(dT, P_, dT, P_).transpose(2, 1, 0, 3))
    wkt = np.ascontiguousarray(np.asarray(W_k, np.float32).T
                               .astype(np.float16))
    wkt_g = np.ascontiguousarray(
        wkt.reshape(dT, P_, dT // 4, 4 * P_).transpose(2, 0, 1, 3))

    in_maps = []
    for c in range(NC):
        xc = np.ascontiguousarray(X_t[:, c * NL:(c + 1) * NL])
        in_maps.append({
            "xt_f32": np.ascontiguousarray(xc.T),
            "xn_f16": np.ascontiguousarray(xc.astype(np.float16)),
            "wkt_g": wkt_g,
            "wvts": np.ascontiguousarray(np.concatenate(
                [wvt[c * P_:(c + 1) * P_, :],
                 wvt[D // 2 + c * P_:D // 2 + (c + 1) * P_, :]], axis=0)),
            "wqt_b": wqt_b,
        })
    return in_maps


_CACHED_NC = None


def _get_nc():
    global _CACHED_NC
    if _CACHED_NC is None:
        _CACHED_NC = build()
    return _CACHED_NC


def run(X_t, W_q, W_k, W_v, trace=False):
    from concourse.bass_utils import run_bass_kernel_spmd
    nc = _get_nc()
    in_maps = prepare_inputs(X_t, W_q, W_k, W_v)
    res = run_bass_kernel_spmd(nc, in_maps, core_ids=list(range(NCORES)),
                               trace=trace)
    A = np.concatenate([res.results[c]["a_out"] for c in range(NCORES)],
                       axis=1)
    return A, res


def kernel(X_t, W_q, W_k, W_v):
    X_t = np.asarray(X_t)
    W_q = np.asarray(W_q)
    W_k = np.asarray(W_k)
    W_v = np.asarray(W_v)
    A, _ = run(X_t, W_q, W_k, W_v, trace=False)
    return A.astype(np.float32)
